# revision 58
# baseline (speedup 1.0000x reference)
"""Multi-head attention (B=2, N=4096, C=512, H=8) on 8 TRN2 NeuronCores.

Sharding: core c handles batch b = c//4 and heads {2*(c%4), 2*(c%4)+1}
(data parallel over B, tensor parallel over heads).  Each core computes its
two heads' full attention plus their slice of the output projection; the
per-core [C, N] projection partials are summed per batch on the host
(the "proj all-reduce") and the projection bias is added there too (with
the attention v-bias folded in: out = Wp(O0 + vb) + pb = Wp O0 + (Wp vb + pb)).

Device-side math per core:
  qT/kT   = Wq/Wk_blk @ x^T + b           [128 (2h x 64d), 4096] fp32 PSUM
  q8/k8   = fp8e4(qT/kT)                  repacked via DRAM roundtrip into
                                          [32 (d0), 2h, 2g, 2t, 8ic, 512] so the
                                          score matmul runs fp8 DoubleRow with
                                          the 64-d contraction split into 2
                                          groups of 32 on partitions 0:32.
  v_nat   = x_jt^T @ Wv  (vb folded out)  [128 (j), 2h x 64d] -> vno fp8
                                          [128, 32jt, (64|1|64|1)] ones cols
  S^T     = DR-matmul(k8, q8)             [128 (j), 512i] fp32 PSUM per head
  P^T     = exp(SCALE * S^T) -> fp8       ACT exp (~60%) or DVE bf16-stage
                                          + GPSIMD pow (~40%), per head-unit
  O_raw^T = fp8 matmul per (unit, head):  lhsT = vno[:, jt, h slice],
            rhs = p2                      -> acc[h] [65, 512] (row 64 = denom)
  O^T     = O_raw^T * (1/denom)           DVE recip + Pool bcast + DVE mult
  out^T  += Wp_blk^T @ O^T                [512, 4096] fp32 -> DRAM
"""

import os
import numpy as np
import ml_dtypes

SC_BUFS = int(os.environ.get("SC_BUFS", "6"))
P2_BUFS = int(os.environ.get("P2_BUFS", "48"))
STG_BUFS = int(os.environ.get("STG_BUFS", "24"))
PDEFER = int(os.environ.get("PDEFER", "21"))
POOL_NUM = int(os.environ.get("POOL_NUM", "3"))   # pool units per POOL_DEN
POOL_DEN = int(os.environ.get("POOL_DEN", "4"))
POOL_NUM0 = int(os.environ.get("POOL_NUM0", "3"))  # ...during i-chunk 0
POOL_DEN0 = int(os.environ.get("POOL_DEN0", "4"))
AB_DIRECT = int(os.environ.get("AB_DIRECT", "1"))
WARM = int(os.environ.get("WARM", "40"))
FLUSH0 = int(os.environ.get("FLUSH0", "32"))  # unit where ic-0 attnV flush starts

B, N, C = 2, 4096, 512
H, Dh = 8, 64
SCALE = Dh**-0.5
NCORES = 8
HPC = 2  # heads per core
ICW = 512  # i-chunk width
NIC = N // ICW  # 8
JTW = 128  # j-tile width
NJT = N // JTW  # 32
NPAIR = NJT // 2  # 16 jt pairs per i-chunk

_BF16 = ml_dtypes.bfloat16
_F8 = ml_dtypes.float8_e4m3

_cached_nc = {}


def _build_nc(reps=1):
    import concourse.bacc as bacc
    import concourse.tile as tile
    import concourse.mybir as mybir

    f32 = mybir.dt.float32
    bf16 = mybir.dt.bfloat16
    fp8 = mybir.dt.float8e4
    Exp = mybir.ActivationFunctionType.Exp
    mult = mybir.AluOpType.mult
    pow_op = mybir.AluOpType.pow
    DR = mybir.MatmulPerfMode.DoubleRow

    nc = bacc.Bacc("TRN2", target_bir_lowering=False, debug=False)

    xt_d = nc.dram_tensor("xt", [C, N], bf16, kind="ExternalInput").ap()
    wqkv_d = nc.dram_tensor("wqkv", [C, 3 * 128], bf16, kind="ExternalInput").ap()
    wp_d = nc.dram_tensor("wp", [128, C], bf16, kind="ExternalInput").ap()
    bqkv_d = nc.dram_tensor("bqkv", [128, 2], f32, kind="ExternalInput").ap()
    out_d = nc.dram_tensor("out", [C, N], f32, kind="ExternalOutput").ap()
    # DRAM scratch for the q8/k8 repack: [d0, h, g, t(q|k), ic, i].  The
    # host permutes the q/k weight columns to (d0, h, g) order so this
    # write is a single contiguous-partition DMA (scores sum over d, so
    # any consistent d-order is exact).
    qk8_d = nc.dram_tensor(
        "qk8s", [32, HPC, 2, 2, NIC, ICW], fp8, kind="ExternalOutput"
    ).ap()

    NUNITS = (N // ICW) * NJT

    def is_pool(idx, h):
        # attnV(u) consumes both heads' exps of unit u: at most one of the
        # two goes Pool so ACT+Pool overlap within the unit.  Final units
        # ACT-only (drain latency).
        if idx >= NUNITS - 3:
            return False
        if idx < NJT:
            sel = (idx * POOL_NUM0) % POOL_DEN0 < POOL_NUM0
        else:
            sel = (idx * POOL_NUM) % POOL_DEN < POOL_NUM
        return sel and h == (idx % 2)

    with tile.TileContext(nc) as tc:
        with (
            tc.tile_pool(name="ps", space="PSUM", bufs=2) as ps,
            tc.tile_pool(name="sp", bufs=2) as sp,
            tc.tile_pool(name="pe", bufs=1) as pe,
        ):
            # --- persistent SBUF tensors ---
            # x^T as one [c0, ct, i] tile: each column piece is a single DMA
            # covering all 4 contraction tiles (one HWDGE slot, not four)
            xt = pe.tile([128, 4, N], bf16, tag="xt", name="xt")

            X_PIECES = [(0, 512), (512, 1536), (1536, 2560), (2560, 4096)]

            def load_x(q0, q1):
                # column pieces: the small first piece unblocks qkv chunk 0
                # fast; later pieces land while the ic-0 unit stream runs.
                for q in range(q0, q1):
                    cs = slice(*X_PIECES[q])
                    nc.sync.dma_start(
                        out=xt[:, :, cs],
                        in_=xt_d.rearrange("(ct p) i -> p ct i", p=128)[:, :, cs],
                    )

            wqkv = pe.tile([128, 4, 3 * 128], bf16, tag="wqkv", name="wqkv")
            wp = pe.tile([128, C], bf16, tag="wp", name="wp")
            bqkv = pe.tile([128, 2], f32, tag="bqkv", name="bqkv")

            def load_w():
                nc.sync.dma_start(
                    out=wqkv[:], in_=wqkv_d.rearrange("(ct p) m -> p ct m", p=128)
                )
                nc.sync.dma_start(out=bqkv[:], in_=bqkv_d[:, :])

            def load_wp():
                nc.sync.dma_start(out=wp[:], in_=wp_d[:, :])

            # q8/k8 in d-split layout: [32 (d0), h, g, t, ic, i]
            qk8 = pe.tile([32, HPC, 2, 2, NIC, ICW], fp8, tag="qk8", name="qk8")
            # v natural layout + ones cols: [j0, jt, (64 v | 1 | 64 v | 1)]
            vno = pe.tile([128, NJT, HPC * (Dh + 1)], fp8, tag="vno", name="vno")
            onorm = pe.tile([128, N], bf16, tag="onorm", name="onorm")
            base = pe.tile([128, ICW], f32, tag="base", name="base")
            nc.vector.memset(base[:], float(np.exp(SCALE)))
            nc.vector.memset(vno[:, :, Dh : Dh + 1], 1.0)
            nc.vector.memset(vno[:, :, 2 * Dh + 1 : 2 * Dh + 2], 1.0)

            # PE pre-warm (pstate ramp) while input DMAs land.
            warm = pe.tile([128, 128], bf16, tag="warm", name="warm")
            nc.vector.memset(warm[:], 0.0)
            wps = ps.tile([128, ICW], f32, tag="sc", bufs=SC_BUFS, name="wps")
            for _ in range(WARM):
                nc.tensor.matmul(
                    wps[:, 0:128], lhsT=warm[:], rhs=warm[:], start=True, stop=True
                )

            # --- QKV production for one 512-column chunk, in parts ---
            qkv_state = {}

            def qkv_qk_part(i, part):
                isl = slice(i * ICW, (i + 1) * ICW)
                if part == 0:
                    qkv_state[i] = []
                qk = ps.tile([128, ICW], f32, tag="sc", bufs=SC_BUFS, name="qk")
                qkv_state[i].append(qk)
                for ct in range(4):
                    nc.tensor.matmul(
                        qk[:],
                        lhsT=wqkv[:, ct, part * 128 : (part + 1) * 128],
                        rhs=xt[:, ct, isl],
                        start=(ct == 0),
                        stop=(ct == 3),
                    )

            def qkv_qk(i):
                qkv_qk_part(i, 0)
                qkv_qk_part(i, 1)

            def qkv_repack(i):
                qk = qkv_state.pop(i)
                qf = sp.tile([128, 2, ICW], fp8, tag="qf", bufs=2, name="qf")
                nc.vector.tensor_scalar_add(
                    out=qf[:, 0, :], in0=qk[0][:], scalar1=bqkv[:, 0:1]
                )
                nc.vector.tensor_scalar_add(
                    out=qf[:, 1, :], in0=qk[1][:], scalar1=bqkv[:, 1:2]
                )
                nc.sync.dma_start(
                    out=qk8_d.rearrange("d h g t ic i -> (d h g) t ic i")[
                        :, :, i, :
                    ],
                    in_=qf[:],
                )

            def qkv_read(i0, i1):
                # read chunks [i0, i1) back from DRAM into the d-split
                # layout: one DMA per t (q/k) covering all (h, g) groups
                for t in range(2):
                    nc.sync.dma_start(
                        out=qk8[:, :, :, t, i0:i1, :],
                        in_=qk8_d[:, :, :, t, i0:i1, :],
                    )

            def qkv_v(i, half):
                if half == 0:
                    vv = ps.tile([128, 4, 128], f32, tag="sc", bufs=SC_BUFS, name="vv")
                    qkv_state[(i, "vv")] = vv
                    rr = range(2)
                else:
                    vv = qkv_state[(i, "vv")]
                    rr = range(2, 4)
                for r in rr:
                    jt = 4 * i + r
                    for ct in range(4):
                        nc.tensor.matmul(
                            vv[:, r, :],
                            lhsT=xt[:, ct, jt * JTW : (jt + 1) * JTW],
                            rhs=wqkv[:, ct, 256:384],
                            start=(ct == 0),
                            stop=(ct == 3),
                        )
                if half == 1:
                    del qkv_state[(i, "vv")]
                    nc.vector.tensor_copy(
                        out=vno[:, 4 * i : 4 * i + 4, :].rearrange(
                            "p r (h dho) -> p r h dho", h=HPC
                        )[:, :, :, 0:Dh],
                        in_=vv[:].rearrange("p r (h d) -> p r h d", h=HPC),
                    )

            # ic-0 interleave schedule: unit -> qkv task
            unit_tasks = {}

            def add_task(u, t):
                unit_tasks.setdefault(u, []).append(t)

            for c in range(3, NIC):
                add_task(3 * (c - 3) + 0, ("q", c))
                add_task(3 * (c - 3) + 1, ("k", c))
                add_task(3 * (c - 3) + 2, ("rp", c))
            add_task(7, ("rd", (3, 5)))
            add_task(13, ("rd", (5, 7)))
            add_task(15, ("rd", (7, 8)))
            for c in range(2, NIC):
                add_task(16 + 2 * (c - 2) + 0, ("vA", c))
                add_task(16 + 2 * (c - 2) + 1, ("vB", c))

            def run_task(task):
                kind, c = task
                if kind == "q":
                    qkv_qk_part(c, 0)
                elif kind == "k":
                    qkv_qk_part(c, 1)
                elif kind == "qk":
                    qkv_qk(c)
                elif kind == "rp":
                    qkv_repack(c)
                elif kind == "rd":
                    qkv_read(*c)
                elif kind == "vA":
                    qkv_v(c, 0)
                else:
                    qkv_v(c, 1)

            def emit_scores(u):
                ic, jt = u
                scs = []
                for h in range(HPC):
                    sc = ps.tile([128, ICW], f32, tag="sc", bufs=SC_BUFS, name="sc")
                    nc.tensor.matmul(
                        sc[:],
                        lhsT=qk8[
                            :, h, :, 1, jt // 4,
                            (jt % 4) * JTW : (jt % 4 + 1) * JTW,
                        ],
                        rhs=qk8[:, h, :, 0, ic, :],
                        start=True,
                        stop=True,
                        perf_mode=DR,
                    )
                    scs.append(sc)
                return scs

            def emit_proj_part(ic, cc, split=False):
                isl = slice(ic * ICW, (ic + 1) * ICW)
                pp = ps.tile([128, ICW], f32, tag="sc", bufs=SC_BUFS, name="pp")
                if split:
                    # per-head-half contraction: part 0 can start as soon as
                    # head 0's normalize mult lands (tail latency)
                    for hh in range(2):
                        nc.tensor.matmul(
                            pp[:],
                            lhsT=wp[hh * 64 : (hh + 1) * 64,
                                    cc * 128 : (cc + 1) * 128],
                            rhs=onorm[hh * 64 : (hh + 1) * 64, isl],
                            start=(hh == 0),
                            stop=(hh == 1),
                        )
                else:
                    nc.tensor.matmul(
                        pp[:],
                        lhsT=wp[:, cc * 128 : (cc + 1) * 128],
                        rhs=onorm[:, isl],
                        start=True,
                        stop=True,
                    )
                st = sp.tile([128, ICW], f32, tag="stl" if split else "st",
                             bufs=4 if split else 2, name="st")
                nc.vector.tensor_copy(out=st[:], in_=pp[:])
                nc.sync.dma_start(
                    out=out_d[cc * 128 : (cc + 1) * 128, isl], in_=st[:]
                )

            # --- attention (software-pipelined over units u = (ic, jt)) ---
            for _rep in range(reps):
                load_w()
                load_x(0, 3)
                # prologue: chunks 0-2 q/k + repack (reads in flight while
                # the v matmuls run), v for chunks 0-1
                qkv_qk(0)
                qkv_repack(0)
                qkv_read(0, 1)
                qkv_qk(1)
                qkv_repack(1)
                qkv_qk(2)
                qkv_repack(2)
                qkv_read(1, 3)
                load_x(3, 4)
                load_wp()
                qkv_v(0, 0)
                qkv_v(0, 1)
                qkv_v(1, 0)
                qkv_v(1, 1)

                # p-state keepalive: filler matmuls into the (still idle)
                # acc0 bank so early DMA-wait gaps don't reset the PE clock
                # ramp back to half speed (acc0 is first allocated at ic-1).
                WARMF = int(os.environ.get("WARMF", "2"))
                warm2 = ps.tile([128, 128], f32, tag="acc0", bufs=1,
                                name="warm2")
                for _ in range(20):
                    nc.tensor.matmul(
                        warm2[:], lhsT=warm[:], rhs=warm[:], start=True,
                        stop=True,
                    )

                units = [(ic, jt) for ic in range(NIC) for jt in range(NJT)]
                accs = {}
                acc_emitted = {}
                p2s = {}
                pending_proj = None
                pending_attn = []

                sc_tiles = {0: emit_scores(units[0])}
                emitted = [0]

                def ensure_scores(upto):
                    while emitted[0] < min(upto, len(units) - 1):
                        emitted[0] += 1
                        sc_tiles[emitted[0]] = emit_scores(units[emitted[0]])

                tasks = []

                def attn_unit(ic, jt):
                    if ic not in accs:
                        accs[ic] = [
                            ps.tile([Dh + 1, ICW], f32, tag=f"acc{h}", bufs=1,
                                    name=f"acc{h}")
                            for h in range(HPC)
                        ]
                        acc_emitted[ic] = 0
                    first = acc_emitted[ic] == 0
                    acc_emitted[ic] += 1
                    last = acc_emitted[ic] == NJT
                    for h in range(HPC):
                        p2 = p2s.pop((ic, jt, h))
                        nc.tensor.matmul(
                            accs[ic][h][0 : Dh + 1, :],
                            lhsT=vno[
                                :, jt,
                                h * (Dh + 1) : (h + 1) * (Dh + 1),
                            ],
                            rhs=p2[:],
                            start=first,
                            stop=last,
                        )
                    if last:
                        # normalize chain, then proj parts, one per unit
                        nc.annotate if False else None
                        tasks.append(("norm", ic, 0))
                        tasks.append(("norm", ic, 1))
                        tasks.append(("norm", ic, 2))
                        tasks.append(("norm", ic, 3))
                        for cc in range(4):
                            tasks.append(("proj", ic, cc))

                def run_tail_task():
                    if not tasks:
                        return
                    kind, ic_, arg = tasks.pop(0)
                    if kind == "norm":
                        normalize_stage(ic_, arg)
                    else:
                        emit_proj_part(ic_, arg, split=(ic_ == NIC - 1))

                norm_state = {}

                def normalize_stage(ic, stage):
                    # 0: recips (DVE), 1: bcasts (Pool), 2: mult h0, 3: mult h1
                    # staged across units so neither DVE nor Pool head-of-line
                    # blocks waiting for the other engine mid-stream.
                    isl = slice(ic * ICW, (ic + 1) * ICW)
                    if stage == 0:
                        rcs = []
                        for h in range(HPC):
                            rc = sp.tile([1, ICW], f32, tag=f"rc{h}", bufs=2,
                                         name="rc")
                            nc.vector.reciprocal(
                                rc[:], accs[ic][h][Dh : Dh + 1, :]
                            )
                            rcs.append(rc)
                        norm_state[ic] = rcs
                    elif stage == 1:
                        rbs = []
                        for h in range(HPC):
                            rb = sp.tile([Dh, ICW], f32, tag=f"rb{h}", bufs=2,
                                         name="rb")
                            nc.gpsimd.partition_broadcast(
                                rb[:], norm_state[ic][h][:]
                            )
                            rbs.append(rb)
                        norm_state[ic] = rbs
                    else:
                        h = stage - 2
                        nc.vector.tensor_tensor(
                            out=onorm[h * Dh : (h + 1) * Dh, isl],
                            in0=accs[ic][h][0:Dh, :],
                            in1=norm_state[ic][h][:],
                            op=mult,
                        )
                        if h == HPC - 1:
                            del norm_state[ic]
                            del accs[ic]

                for idx, (ic, jt) in enumerate(units):
                    scs = sc_tiles.pop(idx)
                    for h in range(HPC):
                        p2 = sp.tile(
                            [128, ICW], fp8, tag="p2", bufs=P2_BUFS, name="p2"
                        )
                        p2s[(ic, jt, h)] = p2
                        sc = scs[h]
                        if is_pool(idx, h):
                            stg = sp.tile(
                                [128, ICW], bf16, tag="stg", bufs=STG_BUFS, name="stg"
                            )
                            nc.vector.tensor_copy(out=stg[:], in_=sc[:])
                            nc.gpsimd.tensor_tensor(
                                out=p2[:], in0=base[:], in1=stg[:], op=pow_op
                            )
                        else:
                            nc.scalar.activation(p2[:], sc[:], Exp, scale=SCALE)
                    # PE: upcoming scores (keeps exp engines fed)
                    ensure_scores(idx + (2 if jt in (NJT - 2, NJT - 1, 0, 1) else 1))
                    # ic-0: interleaved qkv work for chunks 2..7
                    if ic == 0 and jt in unit_tasks:
                        for tk in unit_tasks[jt]:
                            run_task(tk)
                    # one deferred tail task (norm stage / proj part) per unit
                    run_tail_task()
                    if ic == 0 and jt < 24:
                        for _ in range(WARMF):
                            nc.tensor.matmul(
                                warm2[:], lhsT=warm[:], rhs=warm[:],
                                start=True, stop=True,
                            )
                    pending_attn.append((ic, jt))
                    # attnV emission: rate-limited FIFO with carry-over across
                    # chunk boundaries (never bulk-drain: that head-of-line
                    # blocks PE on the freshest exps)
                    if ic == 0 and jt < FLUSH0:
                        pass  # ic-0 units wait for interleaved qkv work
                    else:
                        flushed = 0
                        while pending_attn and (
                            len(pending_attn) > PDEFER
                            or (pending_attn[0][0] < ic and flushed < 4)
                            or (ic == 0 and flushed < 8)
                            or (idx >= NUNITS - 8 and flushed < 4)
                        ):
                            attn_unit(*pending_attn.pop(0))
                            flushed += 1
                # drain: remaining units and tail tasks
                while pending_attn:
                    attn_unit(*pending_attn.pop(0))
                while tasks:
                    run_tail_task()

    nc.compile()
    return nc


def get_nc(reps=1):
    if reps not in _cached_nc:
        _cached_nc[reps] = _build_nc(reps)
    return _cached_nc[reps]


def make_in_maps(x, qkv_w, qkv_b, proj_w):
    """Build the per-core input dicts (host-side sharding + layout prep)."""
    x = np.asarray(x, dtype=np.float32)
    qkv_w = np.asarray(qkv_w, dtype=np.float32)
    qkv_b = np.asarray(qkv_b, dtype=np.float32)
    proj_w = np.asarray(proj_w, dtype=np.float32)

    in_maps = []
    for c in range(NCORES):
        b, j = divmod(c, 4)
        rq = slice(128 * j, 128 * (j + 1))
        rk = slice(512 + 128 * j, 512 + 128 * (j + 1))
        rv = slice(1024 + 128 * j, 1024 + 128 * (j + 1))
        xt = np.ascontiguousarray(x[b].T).astype(_BF16)
        # q/k columns permuted to (d0, h, g) order: p_new = d0*4 + h*2 + g
        # holds original dim h*64 + g*32 + d0 (scores sum over d: exact)
        perm = np.empty(128, dtype=np.int64)
        for d0 in range(32):
            for h in range(HPC):
                for g in range(2):
                    perm[d0 * 4 + h * 2 + g] = h * 64 + g * 32 + d0
        wqkv = np.ascontiguousarray(
            np.concatenate(
                [qkv_w[rq].T[:, perm], qkv_w[rk].T[:, perm], qkv_w[rv].T],
                axis=1,
            )
        ).astype(_BF16)
        wp = np.ascontiguousarray(proj_w[:, rq].T).astype(_BF16)
        bqkv = np.ascontiguousarray(
            np.stack([qkv_b[rq][perm], qkv_b[rk][perm]], axis=1)
        ).astype(np.float32)
        in_maps.append({"xt": xt, "wqkv": wqkv, "wp": wp, "bqkv": bqkv})
    return in_maps


def gather_output(results, proj_b, qkv_b, proj_w):
    """Sum per-core projection partials per batch, transpose, add bias.

    The v-bias is folded in here: out += proj_w @ vb + proj_b.
    """
    proj_b = np.asarray(proj_b, dtype=np.float32)
    vb = np.asarray(qkv_b, dtype=np.float32)[2 * C : 3 * C]
    pb_eff = proj_b + np.asarray(proj_w, dtype=np.float32) @ vb
    out = np.empty((B, N, C), dtype=np.float32)
    for b in range(B):
        acc = np.zeros((C, N), dtype=np.float32)
        for j in range(4):
            acc += np.asarray(results[4 * b + j]["out"], dtype=np.float32)
        out[b] = acc.T + pb_eff
    return out


def kernel(x, qkv_w, qkv_b, proj_w, proj_b):
    from concourse.bass_utils import run_bass_kernel_spmd

    nc = get_nc()
    in_maps = make_in_maps(x, qkv_w, qkv_b, proj_w)
    res = run_bass_kernel_spmd(nc, in_maps, list(range(NCORES)))
    return gather_output(res.results, proj_b, qkv_b, proj_w)


def run_traced(x, qkv_w, qkv_b, proj_w, proj_b, trace_cores=None):
    """Like kernel(), but profiles and returns (out, exec_time_ns, raw result)."""
    from concourse.bass_utils import run_bass_kernel_spmd

    nc = get_nc()
    in_maps = make_in_maps(x, qkv_w, qkv_b, proj_w)
    res = run_bass_kernel_spmd(
        nc, in_maps, list(range(NCORES)), trace=True, trace_cores=trace_cores
    )
    return gather_output(res.results, proj_b, qkv_b, proj_w), res.exec_time_ns, res
